# revision 16
# baseline (speedup 1.0000x reference)
"""DeepSets (MLP + ragged segment-mean) Trainium2 Bass kernel, bf16 edition.

Full inputs in / full outputs out. Data-parallel over sets: tokens are
sharded by contiguous whole-segment ranges across 8 NeuronCores (balanced
by token count), MLP weights replicated, segment-mean local per core.

Per-core device pipeline (feature-major, x pre-transposed to bf16 on host):
  L1  : psum_h1[dh,t] = W1.T @ xT          (TensorE bf16, 2 MMs of N=512)
  evac: h1 = relu(psum + b1) -> bf16 SBUF  (ACT activation / DVE tensor_scalar,
                                            split per EVAC pattern for balance)
  L2  : psum_h2[f,t] = W2.T @ h1           (TensorE bf16, 2-chunk accumulate)
  scan: win = prefix sums of relu(psum+b2) (custom DVE op SCAN_RELU_BIAS_ANT:
                                            fused bias+relu+scan from PSUM,
                                            fp32 state, chained per ITER)
  gath: gpt[slot] = win at segment-end cols (GpSimd ap_gather; cost ~ num_idxs)
  diff: tots = gpt[1:] - gpt[:-1] per window range + window-carry fixup
        (GpSimd extended tensor_tensor, fp32)
  out : transpose 128-slot tiles (TensorE) -> scale by 1/count (ACT) -> DMA
"""

import math
from contextlib import ExitStack

import numpy as np
import ml_dtypes

import concourse.bass as bass
import concourse.tile as tile
from concourse import bacc, mybir
from concourse.bass_utils import run_bass_kernel_spmd
import concourse.dve_ops as dve_ops
from concourse.dve_spec import (
    Spec,
    Src0,
    C0,
    C1,
    relu as dve_relu,
    scan as dve_scan,
    lower as dve_lower,
    AluOp as DveAluOp,
)
from concourse.dve_uop import DveOpSpec


def _register_scan_relu_bias():
    """Fused out[k] = s1 + sum_{j<=k} relu(in0[j] + s0), as one DVE
    instruction reading h2 PSUM directly. Registered at import; sha computed
    from this process's lower() so the golden check is self-consistent."""
    name = "SCAN_RELU_BIAS_ANT"
    for op in dve_ops.OPS:
        if op.name == name:
            return op
    spec = Spec(
        body=dve_scan(DveAluOp.ADD, dve_relu(Src0 + C0), init=C1),
        reference=lambda in0, in1, s0, s1, imm2: (
            np.maximum(in0 + s0, 0).cumsum(axis=-1) + s1
        ),
    )
    row = dve_ops._CUSTOM_DVE_ROW_BASE + len(dve_ops.OPS)
    shas = {}
    for ver in ("v3", "v4"):
        try:
            uops = dve_lower(spec, ver=ver)
            shas[ver] = DveOpSpec(
                name=name, opcode=row, uops=uops, rd1_en=False
            ).sha(ver)
        except Exception:
            pass
    op = dve_ops.DveOp(name, spec, subdim=False, uops_sha=shas)
    dve_ops.OPS.append(op)
    dve_ops.CUSTOM_DVE_SPECS[name] = spec
    dve_ops._SUB_OPCODE_FOR_NAME[name] = row
    return op


SCAN_RELU_BIAS = _register_scan_relu_bias()

N_CORES = 8
D_IN, D_H, D_OUT = 128, 256, 128
WIN = 2048  # tokens per scan/gather window
ITER = 512  # tokens per MLP pipeline iteration (= one psum bank of fp32)
SBUF_BUFS = 3
# per-window count of h1-half evacs on ACT (of 8); cycles through this list;
# the rest go to DVE (which also runs the 4 fused scans per window).
N_ACT_H1 = (6, 7, 6)
# run the small tail ops (win-col0 memset, wsum copy, diff, carry fixup) on
# GpSimd (Pool) to keep DVE/ACT free; fall back to DVE if False.
TAIL_POOL = False
WINP_BUFS = 3
PS3_BUFS = 2
PST_BUFS = 2
# fp8e4m3 DoubleRow for the L2 matmul: one MM with virtual K=256. W2 is
# host-scaled by 64 into fp8 range; b2 scaled to match; invc divided by 64.
FP8_L2 = False
# gather two windows per ap_gather launch (GpSimd launches cost ~2us)
GATHER_PAIR = True

BF16 = mybir.dt.bfloat16
F32 = mybir.dt.float32
FP8 = mybir.dt.float8e4
I16 = mybir.dt.int16
W2_FP8_SCALE = 64.0
NPBF16 = ml_dtypes.bfloat16
RELU = mybir.ActivationFunctionType.Relu
COPY = mybir.ActivationFunctionType.Copy
ADD = mybir.AluOpType.add
SUB = mybir.AluOpType.subtract
MULT = mybir.AluOpType.mult
MAX = mybir.AluOpType.max


def _build_program(t_pad: int, spw: int, n_tr: int, reps: int = 1, mode: str = "full"):
    """Build the single-core SPMD program for t_pad tokens per core.

    spw: gather slots per window (multiple of 16)
    n_tr: number of 128-slot output tiles
    mode: "full" | "dma" | "mm" | "mlp" | "scan" -- ablation timing only;
          non-full modes give wrong results
    """
    n_win = t_pad // WIN
    # gather units: pairs of windows when GATHER_PAIR (last may be solo)
    n_gu = (n_win + 1) // 2 if GATHER_PAIR else n_win
    gu_nidx = (2 * spw) if GATHER_PAIR else spw
    gu16 = gu_nidx // 16
    idxp = ((gu16 + 7) // 8) * 8  # idx block padded to 16B alignment
    g_len = n_tr * 128
    WLEN = (2 * (1 + WIN)) if GATHER_PAIR else (1 + WIN)
    WDT = FP8 if FP8_L2 else BF16

    nc = bacc.Bacc(
        "TRN2", target_bir_lowering=False, debug=False, num_devices=N_CORES
    )
    xT = nc.dram_tensor("xT", [D_IN, t_pad], BF16, kind="ExternalInput").ap()
    w1 = nc.dram_tensor("w1", [D_IN, D_H], BF16, kind="ExternalInput").ap()
    # w2 packed on host: [:, 0:128] = W2[0:128,:], [:, 128:256] = W2[128:256,:]
    w2 = nc.dram_tensor("w2", [128, 2 * D_OUT], WDT, kind="ExternalInput").ap()
    b1 = nc.dram_tensor("b1", [128, 2], F32, kind="ExternalInput").ap()
    b2 = nc.dram_tensor("b2", [128, 1], F32, kind="ExternalInput").ap()
    eye = nc.dram_tensor("eye", [128, 128], F32, kind="ExternalInput").ap()
    gidx = nc.dram_tensor("gidx", [128, n_gu * idxp], I16, kind="ExternalInput").ap()
    invc = nc.dram_tensor("invc", [128, n_tr], F32, kind="ExternalInput").ap()
    out = nc.dram_tensor("out", [g_len, D_OUT], F32, kind="ExternalOutput").ap()

    with tile.TileContext(nc) as tc, ExitStack() as ctx:
        singles = ctx.enter_context(tc.tile_pool(name="singles", bufs=1))
        xin = ctx.enter_context(tc.tile_pool(name="xin", bufs=SBUF_BUFS))
        h1sb = ctx.enter_context(tc.tile_pool(name="h1sb", bufs=SBUF_BUFS))
        winp = ctx.enter_context(tc.tile_pool(name="winp", bufs=WINP_BUFS))
        outp = ctx.enter_context(tc.tile_pool(name="outp", bufs=2))
        ps1 = ctx.enter_context(tc.tile_pool(name="ps1", bufs=2, space="PSUM"))
        ps2 = ctx.enter_context(tc.tile_pool(name="ps2", bufs=2, space="PSUM"))
        ps3 = ctx.enter_context(tc.tile_pool(name="ps3", bufs=PS3_BUFS, space="PSUM"))
        pst_pool = ctx.enter_context(tc.tile_pool(name="pst", bufs=PST_BUFS, space="PSUM"))

        w1s = singles.tile([128, D_H], BF16)
        nc.sync.dma_start(out=w1s[:], in_=w1[:])
        w2s = singles.tile([128, 2 * D_OUT], WDT)
        nc.sync.dma_start(out=w2s[:], in_=w2[:])
        b1s = singles.tile([128, 2], F32)
        nc.sync.dma_start(out=b1s[:], in_=b1[:])
        b2s = singles.tile([128, 1], F32)
        nc.sync.dma_start(out=b2s[:], in_=b2[:])
        eyes = singles.tile([128, 128], F32)
        nc.sync.dma_start(out=eyes[:], in_=eye[:])
        gis = singles.tile([128, n_gu * idxp], I16)
        nc.sync.dma_start(out=gis[:], in_=gidx[:])
        ics = singles.tile([128, n_tr], F32)
        nc.sync.dma_start(out=ics[:], in_=invc[:])
        wsum = singles.tile([128, n_win], F32)
        tots = singles.tile([128, g_len], F32)

        gpt = singles.tile([128, 1 + n_win * spw], F32)
        nc.gpsimd.memset(gpt[:], 0.0)

        def emit_tile(d):
            """Transpose 128 diffed slots to segment-major, scale by 1/count,
            DMA out."""
            pst = pst_pool.tile([128, 128], F32, tag="pst")
            nc.tensor.transpose(pst[:], tots[:, d * 128 : (d + 1) * 128], eyes[:])
            ot = outp.tile([128, 128], F32, tag="ot")
            nc.scalar.activation(ot[:], pst[:], COPY, bias=0.0, scale=ics[:, d : d + 1])
            nc.sync.dma_start(out=out[d * 128 : (d + 1) * 128, :], in_=ot[:])

        tail = nc.gpsimd if TAIL_POOL else nc.vector

        for _rep in range(reps):
          done_tiles = 0
          win = None
          for w in range(n_win):
            xw = xin.tile([128, WIN], BF16, tag="xw")
            nc.sync.dma_start(out=xw[:], in_=xT[:, w * WIN : (w + 1) * WIN])
            if mode == "dma":
                nc.vector.tensor_copy(out=tots[:, 0:1], in_=xw[:, 0:1])
                continue
            n_act_h1 = N_ACT_H1[w % len(N_ACT_H1)]
            if not GATHER_PAIR or w % 2 == 0:
                win = winp.tile([128, WLEN], F32, tag="win")
            off = (w % 2) * (1 + WIN) if GATHER_PAIR else 0
            tail.memset(win[:, off : off + 1], 0.0)
            for it in range(WIN // ITER):
                xt = xw[:, it * ITER : (it + 1) * ITER]
                h1a_ps = ps1.tile([128, ITER], F32, tag="h1a_ps")
                h1b_ps = ps2.tile([128, ITER], F32, tag="h1b_ps")
                nc.tensor.matmul(h1a_ps[:], w1s[:, 0:128], xt, start=True, stop=True)
                nc.tensor.matmul(h1b_ps[:], w1s[:, 128:256], xt, start=True, stop=True)
                if mode == "mm":
                    nc.vector.tensor_copy(out=tots[:, 0:1], in_=h1a_ps[:, 0:1])
                    nc.vector.tensor_copy(out=tots[:, 0:1], in_=h1b_ps[:, 0:1])
                    continue
                if FP8_L2:
                    h1p = h1sb.tile([128, 2 * ITER], FP8, tag="h1p")
                    h1a, h1b = h1p[:, 0:ITER], h1p[:, ITER : 2 * ITER]
                else:
                    h1a_t = h1sb.tile([128, ITER], BF16, tag="h1a")
                    h1b_t = h1sb.tile([128, ITER], BF16, tag="h1b")
                    h1a, h1b = h1a_t[:], h1b_t[:]

                def evac(dst, src, bias_ap, on_act):
                    if on_act:
                        nc.scalar.activation(dst, src[:], RELU, bias=bias_ap)
                    else:
                        nc.vector.tensor_scalar(
                            out=dst,
                            in0=src[:],
                            scalar1=bias_ap,
                            scalar2=0.0,
                            op0=ADD,
                            op1=MAX,
                        )

                # first n_act_h1 of the window's 8 h1-half evacs go to ACT
                evac(h1a, h1a_ps, b1s[:, 0:1], on_act=2 * it < n_act_h1)
                evac(h1b, h1b_ps, b1s[:, 1:2], on_act=2 * it + 1 < n_act_h1)
                h2_ps = ps3.tile([128, ITER], F32, tag="h2_ps")
                if FP8_L2:
                    nc.tensor.matmul(
                        h2_ps[:],
                        w2s[:].rearrange("p (j m) -> p j m", j=2),
                        h1p[:].rearrange("p (j n) -> p j n", j=2),
                        start=True,
                        stop=True,
                        perf_mode=mybir.MatmulPerfMode.DoubleRow,
                    )
                else:
                    nc.tensor.matmul(
                        h2_ps[:], w2s[:, 0:128], h1a, start=True, stop=False
                    )
                    nc.tensor.matmul(
                        h2_ps[:], w2s[:, 128:256], h1b, start=False, stop=True
                    )
                if mode == "mlp":
                    nc.vector.tensor_copy(out=tots[:, 0:1], in_=h2_ps[:, 0:1])
                    continue
                # fused relu(psum+b2) + prefix scan, chained via previous col
                nc.vector._custom_dve(
                    SCAN_RELU_BIAS,
                    out=win[:, off + 1 + it * ITER : off + 1 + (it + 1) * ITER],
                    in0=h2_ps[:],
                    s0=b2s[:, 0:1],
                    s1=win[:, off + it * ITER : off + it * ITER + 1],
                )
            if mode in ("dma", "mm", "mlp"):
                continue
            # window total for the cross-window boundary fixup
            tail.tensor_copy(
                out=wsum[:, w : w + 1], in_=win[:, off + WIN : off + WIN + 1]
            )
            if mode == "scan":
                continue
            if GATHER_PAIR and not (w % 2 == 1 or w == n_win - 1):
                continue
            if GATHER_PAIR:
                g = w // 2
                w0 = 2 * g  # first window of this gather unit
                n_in_unit = w - w0 + 1
                nidx = n_in_unit * spw
                nc.gpsimd.ap_gather(
                    out_ap=gpt[:, 1 + w0 * spw : 1 + (w + 1) * spw],
                    in_ap=win[:] if n_in_unit == 2 else win[:, 0 : WIN + 1],
                    idxs_ap=gis[:, g * idxp : g * idxp + (nidx + 15) // 16],
                    channels=128,
                    num_elems=WLEN if n_in_unit == 2 else WIN + 1,
                    d=1,
                    num_idxs=nidx,
                )
            else:
                w0 = w
                nc.gpsimd.ap_gather(
                    out_ap=gpt[:, 1 + w * spw : 1 + (w + 1) * spw],
                    in_ap=win[:],
                    idxs_ap=gis[:, w * idxp : w * idxp + gu16],
                    channels=128,
                    num_elems=WIN + 1,
                    d=1,
                    num_idxs=spw,
                )
            # diff this unit's slot range into tots
            tail.tensor_tensor(
                out=tots[:, w0 * spw : (w + 1) * spw],
                in0=gpt[:, 1 + w0 * spw : 1 + (w + 1) * spw],
                in1=gpt[:, w0 * spw : (w + 1) * spw],
                op=SUB,
            )
            for wf in range(max(w0, 1), w + 1):
                # first slot of each window's range: add prev window's total
                tail.tensor_tensor(
                    out=tots[:, wf * spw : wf * spw + 1],
                    in0=tots[:, wf * spw : wf * spw + 1],
                    in1=wsum[:, wf - 1 : wf],
                    op=ADD,
                )
            avail = g_len if w == n_win - 1 else ((w + 1) * spw) // 128
            while done_tiles < min(avail, n_tr):
                emit_tile(done_tiles)
                done_tiles += 1

    nc.compile()
    return nc


def _prepare(x, segment_ids, num_segments):
    """Host-side sharding + gather-index construction."""
    n_seg = int(num_segments)
    seg = np.asarray(segment_ids).astype(np.int64)
    T_total = x.shape[0]
    counts = np.bincount(seg, minlength=n_seg).astype(np.int64)
    assert counts.max() < WIN, "segment longer than scan window unsupported"
    cum = np.cumsum(counts)

    # whole-segment split balanced by token count
    split = [0]
    for c in range(1, N_CORES):
        target = c * T_total / N_CORES
        s = int(np.searchsorted(cum, target))
        if s + 1 < n_seg and abs(cum[s] - target) < abs(
            (cum[s - 1] if s > 0 else 0) - target
        ):
            s = s + 1
        s = max(split[-1], min(s, n_seg))
        split.append(s)
    split.append(n_seg)

    cores = []
    max_tok = 1
    for c in range(N_CORES):
        s0, s1 = split[c], split[c + 1]
        t0 = int(cum[s0 - 1]) if s0 > 0 else 0
        t1 = int(cum[s1 - 1]) if s1 > 0 else 0
        cores.append({"s0": s0, "s1": s1, "t0": t0, "t1": t1})
        max_tok = max(max_tok, t1 - t0)

    t_pad = int(math.ceil(max_tok / WIN) * WIN)
    n_win = t_pad // WIN

    # per-core per-window segment-end indices
    max_ends = 1
    for core in cores:
        s0, s1, t0 = core["s0"], core["s1"], core["t0"]
        ends = cum[s0:s1] - 1 - t0  # local end col per segment; may be -1
        win_of = np.maximum(ends, 0) // WIN
        idx_rel = ends - win_of * WIN + 1  # in [0, WIN]
        core["win_of"] = win_of
        core["idx_rel"] = idx_rel
        if len(ends):
            bc = np.bincount(win_of, minlength=n_win)
            max_ends = max(max_ends, int(bc.max()))

    spw = int(math.ceil(max_ends / 16) * 16)
    n_tr = int(math.ceil(n_win * spw / 128))

    for core in cores:
        s0, s1 = core["s0"], core["s1"]
        n_loc = s1 - s0
        slot_of = np.zeros(n_loc, dtype=np.int64)
        idx_full = np.zeros(n_win * spw, dtype=np.int16)
        pos = np.zeros(n_win, dtype=np.int64)
        for j in range(n_loc):
            w = int(core["win_of"][j])
            k = int(pos[w])
            assert k < spw
            idx_full[w * spw + k] = core["idx_rel"][j]
            slot_of[j] = w * spw + k
            pos[w] = k + 1
        for w in range(n_win):
            k = int(pos[w])
            last = idx_full[w * spw + k - 1] if k > 0 else np.int16(0)
            idx_full[w * spw + k : (w + 1) * spw] = last
        core["slot_of"] = slot_of
        # wrap for ap_gather: idxs[j % 16, j // 16] per gather unit, 16B-aligned
        if GATHER_PAIR:
            gu_n = 2 * spw
            idxp = ((gu_n // 16 + 7) // 8) * 8
            blocks = []
            for g in range((n_win + 1) // 2):
                w0 = 2 * g
                arr = idx_full[w0 * spw : (w0 + 1) * spw].astype(np.int32)
                if w0 + 1 < n_win:
                    arr2 = idx_full[(w0 + 1) * spw : (w0 + 2) * spw].astype(np.int32)
                    arr = np.concatenate([arr, arr2 + (WIN + 1)])
                else:
                    arr = np.concatenate([arr, np.zeros(spw, np.int32)])
                blk = np.zeros((16, idxp), dtype=np.int16)
                blk[:, : gu_n // 16] = arr.astype(np.int16).reshape(gu_n // 16, 16).T
                blocks.append(blk)
        else:
            idxp = ((spw // 16 + 7) // 8) * 8
            blocks = []
            for w in range(n_win):
                arr = idx_full[w * spw : (w + 1) * spw]
                blk = np.zeros((16, idxp), dtype=np.int16)
                blk[:, : spw // 16] = arr.reshape(spw // 16, 16).T
                blocks.append(blk)
        gidx16 = np.concatenate(blocks, axis=1)
        core["gidx"] = np.tile(gidx16, (8, 1)).astype(np.int16)
        inv_slot = np.zeros(n_tr * 128, dtype=np.float32)
        inv_slot[slot_of] = 1.0 / np.maximum(counts[s0:s1], 1)
        core["invc"] = np.ascontiguousarray(inv_slot.reshape(n_tr, 128).T)

    return cores, t_pad, spw, n_tr


def _in_maps(x, W1, b1, W2, b2, cores, t_pad):
    w1_np = np.ascontiguousarray(W1.astype(NPBF16))
    w2_pk = np.concatenate([W2[:128, :], W2[128:, :]], axis=1)
    if FP8_L2:
        npfp8 = mybir.dt.np(FP8)
        w2_np = np.ascontiguousarray((w2_pk * W2_FP8_SCALE).astype(npfp8))
        b2_np = np.ascontiguousarray(b2[:, None] * W2_FP8_SCALE)
    else:
        w2_np = np.ascontiguousarray(w2_pk.astype(NPBF16))
        b2_np = np.ascontiguousarray(b2[:, None])
    b1_np = np.ascontiguousarray(np.stack([b1[:128], b1[128:]], axis=1))
    eye_np = np.eye(128, dtype=np.float32)

    in_maps = []
    for core in cores:
        t0, t1 = core["t0"], core["t1"]
        xT_c = np.zeros((D_IN, t_pad), dtype=NPBF16)
        xT_c[:, : t1 - t0] = x[t0:t1].astype(NPBF16).T
        in_maps.append(
            {
                "xT": xT_c,
                "w1": w1_np,
                "w2": w2_np,
                "b1": b1_np,
                "b2": b2_np,
                "eye": eye_np,
                "gidx": core["gidx"],
                "invc": core["invc"] / (W2_FP8_SCALE if FP8_L2 else 1.0),
            }
        )
    return in_maps


_PROGRAM_CACHE = {}


def kernel(x, segment_ids, num_segments, W1, b1, W2, b2):
    x = np.ascontiguousarray(np.asarray(x, dtype=np.float32))
    W1 = np.asarray(W1, dtype=np.float32)
    b1 = np.asarray(b1, dtype=np.float32)
    W2 = np.asarray(W2, dtype=np.float32)
    b2 = np.asarray(b2, dtype=np.float32)
    n_seg = int(num_segments)

    cores, t_pad, spw, n_tr = _prepare(x, segment_ids, num_segments)

    key = (t_pad, spw, n_tr, FP8_L2, GATHER_PAIR)
    if key not in _PROGRAM_CACHE:
        _PROGRAM_CACHE[key] = _build_program(t_pad, spw, n_tr)
    nc = _PROGRAM_CACHE[key]

    in_maps = _in_maps(x, W1, b1, W2, b2, cores, t_pad)
    res = run_bass_kernel_spmd(nc, in_maps, list(range(N_CORES)))

    out_full = np.zeros((n_seg, D_OUT), dtype=np.float32)
    for c, core in enumerate(cores):
        s0, s1 = core["s0"], core["s1"]
        if s1 > s0:
            out_full[s0:s1] = res.results[c]["out"][core["slot_of"]]
    return out_full
